# revision 2
# baseline (speedup 1.0000x reference)
"""Trainium2 Bass kernel for causal top-K (K=8) similarity message passing.

Math per batch b (reference):
  gate  = sigmoid(x @ w_gate + b_gate) * (softplus(log_scale)+0.01)   (host)
  S     = x @ x^T, causal-masked                                      (PE)
  top-8 per row -> tau = 8th value -> sel mask -> indices via a second
  top-8 on sel * (4100 - k)  (vectorized index extraction)
  msg^T = sum of gathered x^T columns (ap_gather + strided reduce)
  out^T = gate * gelu(gain*(mix*x^T + (1-mix)/count*msg^T) + bias)
  y written transposed [dc, dp, q]; host reassembles.

Sharding: 8 cores = 4 batches x 2 query-parity shards. Core c: batch
b=c>>1, parity p=c&1, query tiles g=2t+p (t=0..15) of 128 rows.
Uniform program; all parity dependence lives in per-core input data.
"""

import os
import sys

for _p in ("/opt/trn_rl_repo", os.path.expanduser("~/.axon_site/_ro/trn_rl_repo")):
    if os.path.isdir(_p) and _p not in sys.path:
        sys.path.insert(0, _p)
        break

import numpy as np

import concourse.bacc as bacc
import concourse.mybir as mybir
from concourse.tile import TileContext
from concourse.bass_utils import run_bass_kernel_spmd

F32 = mybir.dt.float32
U16 = mybir.dt.uint16
I16 = mybir.dt.int16
ALU = mybir.AluOpType
AF = mybir.ActivationFunctionType
NEG = np.float32(-1e30)

D = 1024
DC = 8
N_CORES = 8

_prog_cache = {}


def build_program(T, mix, reps=1, stage=4, loop=False):
    """Uniform per-core program. mix is baked as an immediate."""
    local_sim = bool(os.environ.get("BASS_LOCAL_SIM"))
    key = (T, float(mix), reps, local_sim, stage, loop)
    if key in _prog_cache:
        return _prog_cache[key]
    gelu_af = AF.Tanh if local_sim else AF.Gelu  # local interp lacks Gelu

    NQT = T // 256            # query tiles per core
    Q = NQT * 128             # queries per core
    NK = T + 8                # key columns incl zero pad (idx 4096..4103 -> 0)

    nc = bacc.Bacc(trn_type="TRN2", target_bir_lowering=False, debug=False,
                   num_devices=N_CORES)

    xt_in = nc.dram_tensor("xt", [128, DC, NK], F32, kind="ExternalInput").ap()
    xqt_in = nc.dram_tensor("xqt", [NQT, 128, DC, 128], F32,
                            kind="ExternalInput").ap()
    qmask_in = nc.dram_tensor("qmask", [128, 256], F32, kind="ExternalInput").ap()
    gateb_in = nc.dram_tensor("gateb", [128, Q], F32, kind="ExternalInput").ap()
    fix8_in = nc.dram_tensor("fix8", [128, 8], F32, kind="ExternalInput").ap()
    gainc_in = nc.dram_tensor("gainc", [128, DC], F32, kind="ExternalInput").ap()
    biasc_in = nc.dram_tensor("biasc", [128, DC], F32, kind="ExternalInput").ap()
    idxd = nc.dram_tensor("idxd", [NQT * 64, 16], U16, kind="Internal").ap()
    y_out = nc.dram_tensor("y", [DC, 128, Q], F32, kind="ExternalOutput").ap()

    rc8 = float((1.0 - mix) / (8.0 * mix))

    from contextlib import ExitStack

    with TileContext(nc) as tc, ExitStack() as ctx:
        cpool = ctx.enter_context(tc.tile_pool(name="consts", bufs=1))
        xqp = ctx.enter_context(tc.tile_pool(name="xqp", bufs=2))
        smallp = ctx.enter_context(tc.tile_pool(name="smallp", bufs=2))
        wrapp = ctx.enter_context(tc.tile_pool(name="wrapp", bufs=1))
        gathp = ctx.enter_context(tc.tile_pool(name="gathp", bufs=1))
        msgp = ctx.enter_context(tc.tile_pool(name="msgp", bufs=1))
        tmpp = ctx.enter_context(tc.tile_pool(name="tmpp", bufs=1))
        mixp = ctx.enter_context(tc.tile_pool(name="mixp", bufs=1))
        ps_s = ctx.enter_context(tc.tile_pool(name="ps_s", bufs=1, space="PSUM"))

        xt = cpool.tile([128, DC, NK], F32)
        nc.sync.dma_start(out=xt[:], in_=xt_in[:])
        iota = cpool.tile([128, T], I16)
        nc.gpsimd.iota(iota[:], pattern=[[-1, T]], base=4100,
                       channel_multiplier=0)
        qmask = cpool.tile([128, 256], F32)
        nc.sync.dma_start(out=qmask[:], in_=qmask_in[:])
        gateb = cpool.tile([128, Q], F32)
        nc.sync.dma_start(out=gateb[:], in_=gateb_in[:])
        fix8 = cpool.tile([128, 8], F32)
        nc.sync.dma_start(out=fix8[:], in_=fix8_in[:])
        gainc = cpool.tile([128, DC], F32)
        nc.sync.dma_start(out=gainc[:], in_=gainc_in[:])
        biasc = cpool.tile([128, DC], F32)
        nc.sync.dma_start(out=biasc[:], in_=biasc_in[:])

        from contextlib import nullcontext

        def rep_iter():
            if loop:
                return [tc.For_i(0, reps, 1)]
            return [nullcontext(None) for _ in range(reps)]

        for _ctx in rep_iter():
          with _ctx:
            # ---------------- Phase A: scores + top-8 index extraction ------
            for t in range(NQT):
                Lk = (2 * t + 2) * 128
                nblk = (Lk + 511) // 512
                xq = xqp.tile([128, DC, 128], F32, tag="xq", name="xq")
                nc.sync.dma_start(out=xq[:], in_=xqt_in[t])

                S = ps_s.tile([128, T], F32, tag="S", name="S")
                for dc in range(DC):
                    for blk in range(nblk):
                        lo = blk * 512
                        w = min(512, Lk - lo)
                        nc.tensor.matmul(S[:, lo:lo + w], xq[:, dc],
                                         xt[:, dc, lo:lo + w],
                                         start=(dc == 0), stop=(dc == DC - 1),
                                         skip_group_check=True)
                nc.vector.tensor_add(S[:, Lk - 256:Lk], S[:, Lk - 256:Lk],
                                     qmask[:])
                if stage < 2:
                    continue

                v8 = smallp.tile([128, 8], F32, tag="v8", name="v8")
                nc.vector.max(out=v8[:], in_=S[:, :Lk])
                tau = smallp.tile([128, 1], F32, tag="tau", name="tau")
                nc.vector.tensor_scalar(tau[:], v8[:, 7:8], -1e29, None,
                                        op0=ALU.max)
                nc.vector.tensor_scalar(S[:, :Lk], S[:, :Lk], tau[:, 0:1],
                                        None, op0=ALU.is_ge)
                nc.vector.tensor_mul(S[:, :Lk], S[:, :Lk], iota[:, :Lk])
                w8 = smallp.tile([128, 8], F32, tag="w8", name="w8")
                nc.vector.max(out=w8[:], in_=S[:, :Lk])
                idxu = smallp.tile([128, 8], U16, tag="idxu", name="idxu")
                nc.vector.tensor_scalar(idxu[:], w8[:], -1.0, 4100.0,
                                        op0=ALU.mult, op1=ALU.add)
                nc.sync.dma_start(out=idxd[t * 64:(t + 1) * 64, :], in_=idxu[:])

            if stage < 3:
                continue
            # ---------------- wrap: idxd -> [16,1024] replicated x8 ---------
            wrap = wrapp.tile([128, NQT * 64], U16)
            dv = idxd.transpose([1, 0])
            for g in range(8):
                nc.sync.dma_start(out=wrap[g * 16:(g + 1) * 16, :], in_=dv)

            # ---------------- Phase B: gather-aggregate + tail, per dc ------
            for dc in range(DC):
                msgT = msgp.tile([128, Q], F32, tag="msgT", name="msgT")
                for g4 in range(NQT // 4):
                    gath = gathp.tile([128, 4096], F32, tag="gath", name="gath")
                    nc.gpsimd.ap_gather(
                        gath[:], xt[:, dc, :],
                        wrap[:, g4 * 256:(g4 + 1) * 256].bitcast(I16),
                        channels=128, num_elems=NK, d=1, num_idxs=4096)
                    nc.vector.tensor_reduce(
                        msgT[:, g4 * 512:(g4 + 1) * 512],
                        gath[:].rearrange("p (q j) -> p q j", j=8),
                        axis=mybir.AxisListType.X, op=ALU.add)
                if stage < 4:
                    continue

                # tail: out^T = gateb * gelu(gainc*(rc8*msgT*fix + mixa) + biasc)
                nc.vector.tensor_scalar(msgT[:], msgT[:], rc8, None,
                                        op0=ALU.mult)
                nc.vector.tensor_mul(msgT[:, 0:8], msgT[:, 0:8], fix8[:])
                xqsl = mixp.tile([128, Q], F32, tag="xqsl", name="xqsl")
                nc.sync.dma_start(
                    out=xqsl[:].rearrange("p (t q) -> p t q", q=128),
                    in_=xqt_in[:, :, dc, :].transpose([1, 0, 2]))
                nc.vector.tensor_add(msgT[:], msgT[:], xqsl[:])
                t2 = tmpp.tile([128, Q], F32, tag="t2", name="t2")
                nc.scalar.activation(t2[:], msgT[:], gelu_af,
                                     bias=biasc[:, dc:dc + 1],
                                     scale=gainc[:, dc:dc + 1])
                nc.vector.tensor_mul(t2[:], t2[:], gateb[:])
                nc.sync.dma_start(out=y_out[dc], in_=t2[:])

    nc.compile()
    _prog_cache[key] = nc
    return nc


def host_inputs(xb, p, mix, scale, b_gate, w_gate, gain, bias, T):
    """Per-core input arrays for batch slice xb (T,D), parity p."""
    NQT = T // 256
    Q = NQT * 128
    NK = T + 8
    f32 = np.float32
    xb = np.ascontiguousarray(xb, f32)
    xT = xb.T  # (D, T)

    xt = np.zeros((128, DC, NK), f32)
    xt[:, :, :T] = xT.reshape(DC, 128, T).transpose(1, 0, 2)

    rows = ((2 * np.arange(NQT)[:, None] + p) * 128
            + np.arange(128)[None, :]).reshape(-1)        # (Q,) global rows
    xq_cols = xT[:, rows]                                 # (D, Q)
    xqt = np.ascontiguousarray(
        xq_cols.reshape(DC, 128, NQT, 128).transpose(2, 1, 0, 3))

    r = np.arange(128)
    tri_add = np.where(r[None, :] <= r[:, None], f32(0), NEG).astype(f32)
    qmask = np.zeros((128, 256), f32)
    if p == 0:
        qmask[:, :128] = tri_add
        qmask[:, 128:] = NEG
    else:
        qmask[:, 128:] = tri_add

    glin = xb[rows] @ np.asarray(w_gate, f32) + f32(b_gate)
    gate = (1.0 / (1.0 + np.exp(-glin.astype(np.float64)))) * scale
    gateb = np.ascontiguousarray(
        np.broadcast_to(gate.astype(f32)[None, :], (128, Q)))

    fix8 = np.ones((128, 8), f32)
    if p == 0:
        counts = np.minimum(np.arange(8) + 1, 8).astype(f32)
        fix8[:] = (8.0 / counts)[None, :]

    gainc = np.ascontiguousarray(
        (f32(mix) * np.asarray(gain, f32)).reshape(DC, 128).T)
    biasc = np.ascontiguousarray(
        np.asarray(bias, f32).reshape(DC, 128).T)

    return {"xt": xt, "xqt": xqt, "qmask": qmask,
            "gateb": gateb, "fix8": fix8, "gainc": gainc, "biasc": biasc}


def run_cores(x, w_gate, b_gate, gain, bias, log_mix, log_scale, reps=1, stage=4, loop=False):
    x = np.asarray(x, np.float32)
    B, T, _ = x.shape
    mix = float(1.0 / (1.0 + np.exp(-np.float64(log_mix))))
    scale = float(np.logaddexp(0.0, np.float64(log_scale)) + 0.01)
    b_gate_f = float(np.asarray(b_gate, np.float64))

    nc = build_program(T, mix, reps=reps, stage=stage, loop=loop)
    in_maps = []
    for core in range(N_CORES):
        b, p = core >> 1, core & 1
        in_maps.append(host_inputs(x[b % B], p, mix, scale, b_gate_f,
                                   w_gate, gain, bias, T))
    res = run_bass_kernel_spmd(nc, in_maps, list(range(N_CORES)))

    NQT = T // 256
    Q = NQT * 128
    out = np.empty((B, T, D), np.float32)
    for core in range(N_CORES):
        b, p = core >> 1, core & 1
        if b >= B:
            continue
        yc = res.results[core]["y"]                      # (DC, 128, Q)
        yq = yc.transpose(2, 0, 1).reshape(Q, D)         # (Q, D)
        out[b].reshape(T // 128, 128, D)[p::2] = yq.reshape(NQT, 128, D)
    return out


def kernel(x, w_gate, b_gate, gain, bias, log_mix, log_scale, K):
    assert int(K) == 8, "kernel is specialized for K=8"
    return run_cores(x, w_gate, b_gate, gain, bias, log_mix, log_scale)
